# revision 10
# baseline (speedup 1.0000x reference)
"""Trainium2 Bass kernel for Bahdanau-style additive attention.

reference:
  proj_v = values @ W1 + b1              # [B,S,U]
  proj_q = (query @ W2 + b2)[:, None, :] # [B,1,U]
  score  = tanh(proj_v + proj_q) @ V + bV  # [B,S,1]
  aw     = softmax(score, axis=1)
  ctx    = sum(aw * values, axis=1)      # [B,H]

Sharding: batch B=32 across 8 cores (4 batches/core), params replicated.

Per-core dataflow (per batch):
  - values loaded naturally as [128s, H] tiles (kept resident for context pass)
  - PE-transposed to v_T [128h, s] so the big matmul contracts h on partitions
  - proj_T psum tile [128u, 512s] = sum_hc W1_chunk.T @ v_T_chunk (fp32r, N=512)
  - ACT: tanh(psum + pq[u]) fused via per-partition bias
  - score psum [1, 512s] += V_chunk.T @ tanh_tile (accumulated over u chunks)
  - ACT: exp(score + bV) with accum_out giving per-chunk sums; DVE reduces + reciprocal
  - normalized weights written to DRAM output and bounced back as [128s, st] columns
  - ctx psum [1, 512h] += w_col.T @ v_nat_tile accumulated over 16 s-tiles
"""

import numpy as np

B, S, H, U = 32, 2048, 1024, 1024
NCORES = 8
BPC = B // NCORES  # batches per core

P = 128
HC = H // P   # 8 h-chunks
UC = U // P   # 8 u-chunks
ST = S // P   # 16 s-tiles
SCH = 512     # s-chunk (psum free dim)
SC = S // SCH  # 4 s-chunks per batch

_CACHE = {}


def _build():
    import concourse.bass as bass  # noqa: F401
    import concourse.mybir as mybir
    import concourse.tile as tile
    from concourse import bacc
    from concourse.masks import make_identity

    dt = mybir.dt
    f32 = dt.float32
    f32r = dt.float32r
    AF = mybir.ActivationFunctionType

    nc = bacc.Bacc("TRN2", target_bir_lowering=False, debug=False, num_devices=NCORES)

    q_d = nc.dram_tensor("query", [BPC, H], f32, kind="ExternalInput")
    v_d = nc.dram_tensor("values", [BPC, S, H], f32, kind="ExternalInput")
    w1_d = nc.dram_tensor("W1", [H, U], f32, kind="ExternalInput")
    b1_d = nc.dram_tensor("b1", [U], f32, kind="ExternalInput")
    w2_d = nc.dram_tensor("W2", [H, U], f32, kind="ExternalInput")
    b2_d = nc.dram_tensor("b2", [U], f32, kind="ExternalInput")
    V_d = nc.dram_tensor("V", [U, 1], f32, kind="ExternalInput")
    bV_d = nc.dram_tensor("bV", [1], f32, kind="ExternalInput")
    aw_d = nc.dram_tensor("aw", [BPC, S], f32, kind="ExternalOutput")
    ctx_d = nc.dram_tensor("ctx", [BPC, H], f32, kind="ExternalOutput")

    with tile.TileContext(nc) as tc:
        with (
            tc.tile_pool(name="consts", bufs=1) as consts,
            tc.tile_pool(name="w2p", bufs=2) as w2p,
            tc.tile_pool(name="vnat", bufs=20) as vnat_pool,
            tc.tile_pool(name="vt", bufs=2) as vt_pool,
            tc.tile_pool(name="tanhp", bufs=3) as tanh_pool,
            tc.tile_pool(name="rows", bufs=2) as rows,
            tc.tile_pool(name="small", bufs=2) as small,
            tc.tile_pool(name="ps_tr", bufs=2, space="PSUM") as ps_tr,
            tc.tile_pool(name="ps_proj", bufs=3, space="PSUM") as ps_proj,
            tc.tile_pool(name="ps_score", bufs=2, space="PSUM") as ps_score,
            tc.tile_pool(name="ps_ctx", bufs=1, space="PSUM") as ps_ctx,
            tc.tile_pool(name="dramp", bufs=2, space="DRAM") as dram_pool,
        ):
            # ---- constants ----
            ident = consts.tile([P, P], f32)
            make_identity(nc, ident)
            identr = consts.tile([P, P], f32r)
            nc.vector.tensor_copy(identr[:], ident[:])

            w1_sb = consts.tile([P, HC, U], f32r)
            nc.sync.dma_start(
                w1_sb[:], w1_d.rearrange("(hc hp) u -> hp hc u", hp=P).bitcast(f32r)
            )

            qt = consts.tile([P, HC, BPC], f32)
            qt_src = q_d.rearrange("b (hc hp) -> hp hc b", hp=P)
            for hc in range(HC):
                nc.sync.dma_start(qt[:, hc, :], qt_src[:, hc, :])

            b1t = small.tile([P, UC], f32)
            nc.sync.dma_start(b1t[:], b1_d.rearrange("(uc up) -> up uc", up=P))
            b2t = small.tile([P, UC], f32)
            nc.sync.dma_start(b2t[:], b2_d.rearrange("(uc up) -> up uc", up=P))
            bsum = consts.tile([P, UC], f32)
            nc.vector.tensor_add(bsum[:], b1t[:], b2t[:])

            v_sb = consts.tile([P, UC], f32r)
            nc.sync.dma_start(
                v_sb[:], V_d.rearrange("(uc up) one -> up (uc one)", up=P).bitcast(f32r)
            )
            bv_sb = consts.tile([1, 1], f32)
            nc.sync.dma_start(bv_sb[:], bV_d.rearrange("(a one) -> a one", a=1))

            # ---- pq[u, b] = (query @ W2)[b, u] + b1[u] + b2[u] ----
            # out rows [4b, 1024u] accumulated over h-chunks, then transposed
            pq = consts.tile([P, UC, BPC], f32)
            pqr_ps = [
                ps_tr.tile([BPC, SCH], f32, tag="tr", name=f"pqr_ps{i}")
                for i in range(2)
            ]
            w2_tiles = []
            for hc in range(HC):
                w2t = w2p.tile([P, U], f32, tag="w2")
                nc.sync.dma_start(w2t[:], w2_d[hc * P : (hc + 1) * P, :])
                w2_tiles.append(w2t)
                for half in range(2):
                    nc.tensor.matmul(
                        pqr_ps[half][:],
                        qt[:, hc, :],
                        w2t[:, half * SCH : (half + 1) * SCH],
                        start=(hc == 0),
                        stop=(hc == HC - 1),
                    )
            pq_rows = small.tile([BPC, U], f32)
            for half in range(2):
                nc.vector.tensor_copy(
                    pq_rows[:, half * SCH : (half + 1) * SCH], pqr_ps[half][:]
                )
            for uc in range(UC):
                pqT_ps = ps_tr.tile([P, BPC], f32, tag="tr")
                nc.tensor.transpose(
                    pqT_ps[:], pq_rows[:, uc * P : (uc + 1) * P], ident[:BPC, :BPC]
                )
                nc.vector.tensor_scalar_add(
                    pq[:, uc, :], pqT_ps[:], bsum[:, uc : uc + 1]
                )

            # ---- main per-batch loop ----
            for b in range(BPC):
                # load values naturally: 16 tiles [128s, H]
                vn = []
                for st in range(ST):
                    t = vnat_pool.tile([P, H], f32r, tag="vn")
                    nc.sync.dma_start(t[:], v_d[b, st * P : (st + 1) * P, :].bitcast(f32r))
                    vn.append(t)

                score_ps = []
                for sc in range(SC):
                    # transpose the 4 s-subtiles of this chunk: v_T [128h, hc, 512s]
                    vt = vt_pool.tile([P, HC, SCH], f32r, tag="vt")
                    for hc in range(HC):
                        tr_ps = ps_tr.tile([P, SCH], f32r, tag="tr")
                        for q in range(SCH // P):
                            st = sc * (SCH // P) + q
                            nc.tensor.transpose(
                                tr_ps[:, q * P : (q + 1) * P],
                                vn[st][:, hc * P : (hc + 1) * P],
                                identr[:],
                            )
                        if hc % 2 == 0:
                            nc.vector.tensor_copy(vt[:, hc, :], tr_ps[:])
                        else:
                            nc.scalar.copy(vt[:, hc, :], tr_ps[:])

                    # proj + tanh + score over u-chunks
                    s_ps = ps_score.tile([1, SCH], f32, tag="score")
                    score_ps.append(s_ps)
                    for uc in range(UC):
                        p_ps = ps_proj.tile([P, SCH], f32, tag="proj")
                        for hc in range(HC):
                            nc.tensor.matmul(
                                p_ps[:],
                                w1_sb[:, hc, uc * P : (uc + 1) * P],
                                vt[:, hc, :],
                                start=(hc == 0),
                                stop=(hc == HC - 1),
                            )
                        th = tanh_pool.tile([P, SCH], f32r, tag="tanh")
                        nc.scalar.activation(
                            th[:], p_ps[:], AF.Tanh, bias=pq[:, uc, b : b + 1]
                        )
                        nc.tensor.matmul(
                            s_ps[:],
                            v_sb[:, uc : uc + 1],
                            th[:],
                            start=(uc == 0),
                            stop=(uc == UC - 1),
                        )

                # softmax over S (no max subtraction: |score| <= ||V||_1, small)
                exp_sb = rows.tile([1, S], f32, tag="exp")
                acc = small.tile([1, SC], f32, tag="acc")
                for sc in range(SC):
                    nc.scalar.activation(
                        exp_sb[0:1, sc * SCH : (sc + 1) * SCH],
                        score_ps[sc][:],
                        AF.Exp,
                        bias=bv_sb[0:1, 0:1],
                        accum_out=acc[0:1, sc : sc + 1],
                    )
                z = small.tile([1, 1], f32, tag="z")
                nc.vector.tensor_reduce(
                    z[:], acc[:], axis=mybir.AxisListType.X, op=mybir.AluOpType.add
                )
                invz = small.tile([1, 1], f32, tag="invz")
                nc.vector.reciprocal(invz[:], z[:])
                # normalize in place
                nc.vector.tensor_scalar_mul(exp_sb[:], exp_sb[:], invz[0:1, 0:1])
                # attention-weights output
                nc.sync.dma_start(aw_d[b : b + 1, :], exp_sb[:])
                # bounce through DRAM to get weights as [128s, st] columns
                scr = dram_pool.tile([1, S], f32, tag="scr")
                nc.sync.dma_start(scr[:], exp_sb[:])
                w_cols = small.tile([P, ST], f32r, tag="wcols")
                nc.sync.dma_start(
                    w_cols[:],
                    scr[:].rearrange("one (st sp) -> (one sp) st", sp=P).bitcast(f32r)
                )

                # context: ctx[h] = sum_s w[s] * values[s, h]
                ctx_sb = small.tile([1, H], f32, tag="ctxrow")
                for h2 in range(H // SCH):
                    c_ps = ps_ctx.tile([1, SCH], f32, tag="ctx")
                    for st in range(ST):
                        nc.tensor.matmul(
                            c_ps[:],
                            w_cols[:, st : st + 1],
                            vn[st][:, h2 * SCH : (h2 + 1) * SCH],
                            start=(st == 0),
                            stop=(st == ST - 1),
                        )
                    nc.scalar.copy(ctx_sb[0:1, h2 * SCH : (h2 + 1) * SCH], c_ps[:])
                nc.sync.dma_start(ctx_d[b : b + 1, :], ctx_sb[:])

    nc.compile()
    return nc


def _get_nc():
    if "nc" not in _CACHE:
        _CACHE["nc"] = _build()
    return _CACHE["nc"]


def kernel(**inputs):
    from concourse.bass_utils import run_bass_kernel_spmd

    nc = _get_nc()

    q = np.ascontiguousarray(np.asarray(inputs["query"], dtype=np.float32))
    v = np.ascontiguousarray(np.asarray(inputs["values"], dtype=np.float32))
    shared = {
        k: np.ascontiguousarray(np.asarray(inputs[k], dtype=np.float32))
        for k in ("W1", "b1", "W2", "b2", "V", "bV")
    }

    in_maps = []
    for c in range(NCORES):
        sl = slice(c * BPC, (c + 1) * BPC)
        in_maps.append(
            {
                "query": np.ascontiguousarray(q[sl]),
                "values": np.ascontiguousarray(v[sl]),
                **shared,
            }
        )

    res = run_bass_kernel_spmd(nc, in_maps, core_ids=list(range(NCORES)))
    _CACHE["last_res"] = res
    aw = np.concatenate([r["aw"] for r in res.results], axis=0).reshape(B, S, 1)
    ctx = np.concatenate([r["ctx"] for r in res.results], axis=0)
    return aw.astype(np.float32), ctx.astype(np.float32)
